# revision 2
# baseline (speedup 1.0000x reference)
"""Raw-bacc MaxPool3d kernel v10: v6 + loads/stores alternated across
both HWDGE rows (SP and ACT).

Per-core trace analysis shows a roaming "cursed" (row, engine) DMA ring
cell on this part: one SDMA engine runs HBM->SBUF loads ~24% slow on
one (sometimes both) of the two HWDGE rows, and which core/engine/row
is affected drifts between runs. A uniform one-row load stream
throttles the whole core to that engine. Alternating tile loads between
the SP row and the ACT row halves the bytes exposed to any row-bound
slow cell (engine-bound cells are unaffected either way), without
hurting clean cores.

Same tile geometry as v6: tile = half channel, partition p = 4d' + hb,
a0/a1 even/odd plane buffers, pool h-pairs then w-pairs on DVE, combine
for the d-pair. Every DMA spans 128 partitions (16 sem increments).
"""

import numpy as np

import concourse.bass as bass
from concourse import bacc, mybir
from concourse import bass_utils

CPC = 8
D = H = W = 128
DT = mybir.dt.float32
NSLOT = 3
NT = 16

_CACHE = {}


def _build_module():
    nc = bacc.Bacc("TRN2", target_bir_lowering=False, debug=False, num_devices=8)
    x = nc.dram_tensor("x", [CPC, D, H, W], DT, kind="ExternalInput").ap()
    y = nc.dram_tensor("y", [CPC, D // 2, H // 2, W // 2], DT, kind="ExternalOutput").ap()

    a0 = [nc.alloc_sbuf_tensor(f"a0_{i}", [128, 32, 128], DT).ap() for i in range(NSLOT)]
    a1 = [nc.alloc_sbuf_tensor(f"a1_{i}", [128, 32, 128], DT).ap() for i in range(NSLOT)]
    hm = nc.alloc_sbuf_tensor("hm", [128, 16, 128], DT).ap()
    b0 = [nc.alloc_sbuf_tensor(f"b0_{i}", [128, 16, 64], DT).ap() for i in range(2)]
    b1 = nc.alloc_sbuf_tensor("b1", [128, 16, 64], DT).ap()
    wm = [nc.alloc_sbuf_tensor(f"wm_{i}", [128, 16, 64], DT).ap() for i in range(2)]

    a0_sems = [nc.alloc_semaphore(f"a0_sem{i}") for i in range(NSLOT)]
    a1_sems = [nc.alloc_semaphore(f"a1_sem{i}") for i in range(NSLOT)]
    wm_sems = [nc.alloc_semaphore(f"wm_sem{i}") for i in range(2)]
    a1hi_sem = nc.alloc_semaphore("a1hi_sem")
    rel_sem = nc.alloc_semaphore("rel_sem")
    comp_sem = nc.alloc_semaphore("comp_sem")

    def tile_slices(t):
        c, half = divmod(t, 2)
        return c, half * 64

    # loads on SP for even tiles / ACT for odd tiles; stores the opposite.
    # Both engines issue their tiles' loads in program order; every DMA is
    # 128-partition so each .then_inc delivers exactly 16 increments.
    def ldeng(t):
        return nc.sync if t % 2 == 0 else nc.scalar

    def steng(t):
        return nc.scalar if t % 2 == 0 else nc.sync

    # --- loads + stores, interleaved per tile -------------------------
    # Emission order shapes each engine's instruction stream: iteration t
    # emits tile t's loads on ldeng(t) and tile (t-2)'s store on
    # steng(t-2) (the other engine), so each engine's own stream is
    # [.., load t, store t-1, load t+2, ..] -- stores never block loads
    # more than ~2 tiles ahead and there is no cross-engine deadlock.
    def emit_load(t):
        c, base = tile_slices(t)
        k = t % NSLOT
        eng = ldeng(t)
        if t >= NSLOT:
            eng.wait_ge(rel_sem, t - NSLOT + 1)
        eng.dma_start(a0[k], x[c, base : base + 64 : 2]).then_inc(a0_sems[k], 16)
        odd = x[c, base + 1 : base + 64 : 2]
        if t < NT - 1:
            eng.dma_start(a1[k], odd).then_inc(a1_sems[k], 16)
        else:
            # final tile: split the odd-plane load so DVE can pool the
            # first half while the second half is still in flight.
            oddr = odd.rearrange("d (hb r) w -> d hb (r w)", hb=4)
            eng.dma_start(a1[k][:, 0:16, :], oddr[:, :, 0:2048]).then_inc(
                a1_sems[k], 16
            )
            eng.dma_start(a1[k][:, 16:32, :], oddr[:, :, 2048:4096]).then_inc(
                a1hi_sem, 16
            )

    def emit_store(t):
        c, base = tile_slices(t)
        m = t % 2
        eng = steng(t)
        eng.wait_ge(comp_sem, t + 1)
        eng.dma_start(y[c, base // 2 : base // 2 + 32], wm[m]).then_inc(
            wm_sems[m], 16
        )

    for t in range(NT):
        emit_load(t)
        if t >= 2:
            emit_store(t - 2)
    emit_store(NT - 2)
    emit_store(NT - 1)
    nc.scalar.wait_ge(wm_sems[0], 16 * (NT // 2))
    nc.scalar.wait_ge(wm_sems[1], 16 * (NT // 2))
    nc.sync.wait_ge(wm_sems[0], 16 * (NT // 2))
    nc.sync.wait_ge(wm_sems[1], 16 * (NT // 2))

    # --- DVE -----------------------------------------------------------
    def pool_hw(dst, src, nrow=32):
        hv = hm[:, 0 : nrow // 2, :]
        nc.vector.tensor_max(hv, src[:, 0::2, :], src[:, 1::2, :])
        wp = hv.rearrange("p r (w2 two) -> p r w2 two", two=2)
        return nc.vector.tensor_max(dst, wp[:, :, :, 0], wp[:, :, :, 1])

    wm_uses = [0, 0]
    for t in range(NT):
        k = t % NSLOT
        m = t % 2
        uses = t // NSLOT + 1
        nc.vector.wait_ge(a0_sems[k], 16 * uses)
        pool_hw(b0[m], a0[k])
        nc.vector.wait_ge(a1_sems[k], 16 * uses)
        if t < NT - 1:
            pool_hw(b1, a1[k]).then_inc(rel_sem, 1)
        else:
            pool_hw(b1[:, 0:8, :], a1[k][:, 0:16, :], 16)
            nc.vector.wait_ge(a1hi_sem, 16)
            pool_hw(b1[:, 8:16, :], a1[k][:, 16:32, :], 16).then_inc(rel_sem, 1)
        if wm_uses[m] > 0:
            nc.vector.wait_ge(wm_sems[m], 16 * wm_uses[m])
        nc.vector.tensor_max(wm[m], b0[m], b1).then_inc(comp_sem, 1)
        wm_uses[m] += 1

    nc.compile()
    return nc


def _get_module():
    if "nc" not in _CACHE:
        _CACHE["nc"] = _build_module()
    return _CACHE["nc"]


def kernel(x: np.ndarray) -> np.ndarray:
    B, C, d, h, w = x.shape
    assert (B, C, d, h, w) == (2, 32, 128, 128, 128), x.shape
    nc = _get_module()

    xf = np.ascontiguousarray(x, dtype=np.float32).reshape(B * C, d, h, w)
    in_maps = [
        {"x": np.ascontiguousarray(xf[i * CPC : (i + 1) * CPC])} for i in range(8)
    ]
    res = bass_utils.run_bass_kernel_spmd(nc, in_maps, core_ids=list(range(8)))
    out = np.concatenate([r["y"] for r in res.results], axis=0)
    return out.reshape(B, C, d // 2, h // 2, w // 2)


# revision 4
# speedup vs baseline: 1.6704x; 1.6704x over previous
"""MaxPool3d kernel v16: v12 + every parity load split into row-halves.

Each 4 MiB parity load becomes two 2 MiB half-loads (rows [0,32) and
[32,64)) with separate semaphores: DVE pools the low half while the
high half is still streaming, the slot-release fires ~2 us earlier
(fewer bubbles when a slow SDMA engine stretches the tile), and the
final-tile tail split becomes uniform across all tiles.

rel0/rel1 release the a0/a1 halves of a slot independently: the even-
plane buffer of tile t is free ~half a tile earlier than the odd one,
so the t+2 even load is enqueued long before the ring drains (NSLOT=2
full-channel tiles otherwise bubble ~1 us per tile on slot recycle).
The last tile's combine and store run in h-halves to shorten the tail.

The roaming slow-SDMA-engine pathology costs cursed engines ~150 ns per
16 KiB load descriptor (measured 730 vs 580 ns). If that penalty is
per-descriptor (not per-byte), halving the descriptor count by doubling
descriptor size to 32 KiB (the max below the 64 KiB SDMA limit) halves
the damage: cursed cores ~205 us instead of ~235. Bigger DMAs (4 MiB)
also sit higher on the DMA efficiency curve for clean cores.

Tile = one full channel: partition p = 2*d'' + hh holds rows
[64hh, 64hh+64) of plane 2d''+par -- one contiguous 32 KiB chunk per
partition per parity. Pool h-pairs then w-pairs on DVE; even/odd plane
buffers give the d-pair. Loads alternate SP/ACT rows per tile (halves
exposure to row-bound slow cells); stores ride the opposite row,
emitted two tiles behind loads to keep both sequencer streams deep.
"""

import numpy as np

import concourse.bass as bass
from concourse import bacc, mybir
from concourse import bass_utils

CPC = 8
D = H = W = 128
DT = mybir.dt.float32
NSLOT = 2
NT = 8

_CACHE = {}


def _build_module():
    nc = bacc.Bacc("TRN2", target_bir_lowering=False, debug=False, num_devices=8)
    x = nc.dram_tensor("x", [CPC, D, H, W], DT, kind="ExternalInput").ap()
    y = nc.dram_tensor("y", [CPC, D // 2, H // 2, W // 2], DT, kind="ExternalOutput").ap()

    a0 = [nc.alloc_sbuf_tensor(f"a0_{i}", [128, 64, 128], DT).ap() for i in range(NSLOT)]
    a1 = [nc.alloc_sbuf_tensor(f"a1_{i}", [128, 64, 128], DT).ap() for i in range(NSLOT)]
    hm = nc.alloc_sbuf_tensor("hm", [128, 32, 128], DT).ap()
    b0 = [nc.alloc_sbuf_tensor(f"b0_{i}", [128, 32, 64], DT).ap() for i in range(2)]
    b1 = nc.alloc_sbuf_tensor("b1", [128, 32, 64], DT).ap()
    wm = [nc.alloc_sbuf_tensor(f"wm_{i}", [128, 32, 64], DT).ap() for i in range(2)]

    a0lo_sems = [nc.alloc_semaphore(f"a0lo_sem{i}") for i in range(NSLOT)]
    a0hi_sems = [nc.alloc_semaphore(f"a0hi_sem{i}") for i in range(NSLOT)]
    a1lo_sems = [nc.alloc_semaphore(f"a1lo_sem{i}") for i in range(NSLOT)]
    a1hi_sems = [nc.alloc_semaphore(f"a1hi_sem{i}") for i in range(NSLOT)]
    wm_sems = [nc.alloc_semaphore(f"wm_sem{i}") for i in range(2)]
    rel0_sem = nc.alloc_semaphore("rel0_sem")
    rel1_sem = nc.alloc_semaphore("rel1_sem")
    comp_sem = nc.alloc_semaphore("comp_sem")

    def ldeng(t):
        return nc.sync if t % 2 == 0 else nc.scalar

    def steng(t):
        return nc.scalar if t % 2 == 0 else nc.sync

    def emit_load(t):
        k = t % NSLOT
        eng = ldeng(t)
        Be = x[t, 0:D:2].rearrange("d (hh r) w -> d hh r w", hh=2)
        Bo = x[t, 1:D:2].rearrange("d (hh r) w -> d hh r w", hh=2)
        if t >= NSLOT:
            eng.wait_ge(rel0_sem, t - NSLOT + 1)
        eng.dma_start(a0[k][:, 0:32, :], Be[:, :, 0:32, :]).then_inc(
            a0lo_sems[k], 16
        )
        eng.dma_start(a0[k][:, 32:64, :], Be[:, :, 32:64, :]).then_inc(
            a0hi_sems[k], 16
        )
        if t >= NSLOT:
            eng.wait_ge(rel1_sem, t - NSLOT + 1)
        eng.dma_start(a1[k][:, 0:32, :], Bo[:, :, 0:32, :]).then_inc(
            a1lo_sems[k], 16
        )
        eng.dma_start(a1[k][:, 32:64, :], Bo[:, :, 32:64, :]).then_inc(
            a1hi_sems[k], 16
        )

    def emit_store(t):
        m = t % 2
        eng = steng(t)
        if t < NT - 1:
            eng.wait_ge(comp_sem, 2 * t + 2)
            eng.dma_start(y[t], wm[m]).then_inc(wm_sems[m], 16)
        else:
            # final tile: combines land in h-halves (comp +2); store each
            # half as soon as its combine is done to shorten the tail
            yv = y[t].rearrange("d (hh r) w -> d hh r w", hh=2)
            eng.wait_ge(comp_sem, 2 * t + 1)
            eng.dma_start(yv[:, :, 0:16, :], wm[m][:, 0:16, :]).then_inc(
                wm_sems[m], 16
            )
            eng.wait_ge(comp_sem, 2 * t + 2)
            eng.dma_start(yv[:, :, 16:32, :], wm[m][:, 16:32, :]).then_inc(
                wm_sems[m], 16
            )

    for t in range(NT):
        emit_load(t)
        if t >= 2:
            emit_store(t - 2)
    emit_store(NT - 2)
    emit_store(NT - 1)
    nc.scalar.wait_ge(wm_sems[0], 16 * (NT // 2))
    nc.scalar.wait_ge(wm_sems[1], 16 * (NT // 2 + 1))
    nc.sync.wait_ge(wm_sems[0], 16 * (NT // 2))
    nc.sync.wait_ge(wm_sems[1], 16 * (NT // 2 + 1))

    # --- DVE -----------------------------------------------------------
    def pool_hw(dst, src, nrow=64):
        hv = hm[:, 0 : nrow // 2, :]
        nc.vector.tensor_max(hv, src[:, 0:nrow:2, :], src[:, 1:nrow:2, :])
        wp = hv.rearrange("p r (w2 two) -> p r w2 two", two=2)
        return nc.vector.tensor_max(dst, wp[:, :, :, 0], wp[:, :, :, 1])

    wm_uses = [0, 0]
    for t in range(NT):
        k = t % NSLOT
        m = t % 2
        uses = t // NSLOT + 1
        nc.vector.wait_ge(a0lo_sems[k], 16 * uses)
        pool_hw(b0[m][:, 0:16, :], a0[k][:, 0:32, :], 32)
        nc.vector.wait_ge(a0hi_sems[k], 16 * uses)
        pool_hw(b0[m][:, 16:32, :], a0[k][:, 32:64, :], 32).then_inc(rel0_sem, 1)
        nc.vector.wait_ge(a1lo_sems[k], 16 * uses)
        pool_hw(b1[:, 0:16, :], a1[k][:, 0:32, :], 32)
        if wm_uses[m] > 0:
            nc.vector.wait_ge(wm_sems[m], 16 * wm_uses[m])
        nc.vector.tensor_max(
            wm[m][:, 0:16, :], b0[m][:, 0:16, :], b1[:, 0:16, :]
        ).then_inc(comp_sem, 1)
        nc.vector.wait_ge(a1hi_sems[k], 16 * uses)
        pool_hw(b1[:, 16:32, :], a1[k][:, 32:64, :], 32).then_inc(rel1_sem, 1)
        nc.vector.tensor_max(
            wm[m][:, 16:32, :], b0[m][:, 16:32, :], b1[:, 16:32, :]
        ).then_inc(comp_sem, 1)
        wm_uses[m] += 1

    nc.compile()
    return nc


def _get_module():
    if "nc" not in _CACHE:
        _CACHE["nc"] = _build_module()
    return _CACHE["nc"]


def kernel(x: np.ndarray) -> np.ndarray:
    B, C, d, h, w = x.shape
    assert (B, C, d, h, w) == (2, 32, 128, 128, 128), x.shape
    nc = _get_module()

    xf = np.ascontiguousarray(x, dtype=np.float32).reshape(B * C, d, h, w)
    in_maps = [
        {"x": np.ascontiguousarray(xf[i * CPC : (i + 1) * CPC])} for i in range(8)
    ]
    res = bass_utils.run_bass_kernel_spmd(nc, in_maps, core_ids=list(range(8)))
    out = np.concatenate([r["y"] for r in res.results], axis=0)
    return out.reshape(B, C, d // 2, h // 2, w // 2)
